# revision 6
# baseline (speedup 1.0000x reference)
"""Trainium2 Bass kernel for local sparse attention.

Reference computation (b=4, n=8192, k=16, d=128):
    Q = src @ Wq.T                                   [b,n,d]
    K = tgt @ Wk.T ; V = tgt @ Wv.T                  [b,n,k,d]
    scores = einsum('bnkd,bnd->bnk', K, Q) / sqrt(d)
    attn = softmax(scores, axis=-1)
    out = einsum('bnk,bnkd->bnd', attn, V)

Algebraic restructuring (the key to reaching the memory roofline):
    scores[n,k] = tgt[n,k,:] . (src[n,:] @ (Wq.T @ Wk) / sqrt(d))
    out[n,:]    = (sum_k attn[n,k] * tgt[n,k,:]) @ Wv.T
so the huge K/V projections (34 GFLOP) are never materialized; the kernel
streams tgt once from HBM (256 MiB total) and does only small matmuls.

Sharding: data-parallel over the flattened (b*n) = 32768 points across
8 NeuronCores; 4096 points per core; attention is fully local per point.
"""

import math

import numpy as np

# ---- problem constants (hardcoded per the contract) ----
B, N, KNBR, D = 4, 8192, 16, 128
NCORES = 8
PTS_TOTAL = B * N              # 32768
PTS_CORE = PTS_TOTAL // NCORES  # 4096
TILE_PTS = 128                 # points per inner tile
NTILES = PTS_CORE // TILE_PTS  # 32
GRP = TILE_PTS // (128 // KNBR)  # 16 groups of 8 points per tile
PTS_G = 128 // KNBR            # 8 points per group

_cached = {}


def _build_program(pts_core=PTS_CORE, num_devices=NCORES):
    import concourse.bacc as bacc
    import concourse.tile as tile
    from concourse import mybir

    NTILES = pts_core // TILE_PTS

    nc = bacc.Bacc("TRN2", target_bir_lowering=False, debug=False,
                   num_devices=num_devices)

    f32 = mybir.dt.float32
    src_h = nc.dram_tensor("src_sh", [pts_core, D], f32, kind="ExternalInput").ap()
    tgt_h = nc.dram_tensor("tgt_sh", [pts_core * KNBR, D], f32, kind="ExternalInput").ap()
    wqk_h = nc.dram_tensor("wqk", [D, D], f32, kind="ExternalInput").ap()
    wvt_h = nc.dram_tensor("wvt", [D, D], f32, kind="ExternalInput").ap()
    iden_h = nc.dram_tensor("iden", [D, D], f32, kind="ExternalInput").ap()
    out_h = nc.dram_tensor("out_sh", [pts_core, D], f32, kind="ExternalOutput").ap()

    with tile.TileContext(nc) as tc:
        with (
            tc.tile_pool(name="consts", bufs=1) as consts,
            tc.tile_pool(name="big", bufs=2) as big,
            tc.tile_pool(name="small", bufs=3) as small,
            tc.tile_pool(name="ps", bufs=2, space="PSUM") as ps,
            tc.tile_pool(name="ps_ctx", bufs=2, space="PSUM") as ps_ctx,
        ):
            wqk_sb = consts.tile([D, D], f32)
            nc.sync.dma_start(out=wqk_sb, in_=wqk_h)
            wvt_sb = consts.tile([D, D], f32)
            nc.sync.dma_start(out=wvt_sb, in_=wvt_h)
            iden_sb = consts.tile([D, D], f32)
            nc.sync.dma_start(out=iden_sb, in_=iden_h)

            for t in range(NTILES):
                # ---- load tgt tile in "q-layout": 16 group-tiles, each 128
                # consecutive HBM rows (64KB contiguous) = [q=(j,k), d]
                t2 = big.tile([128, GRP, D], f32, tag="t2")
                row0 = t * TILE_PTS * KNBR
                for g in range(GRP):
                    nc.sync.dma_start(
                        out=t2[:, g, :],
                        in_=tgt_h[row0 + g * 128: row0 + (g + 1) * 128, :],
                    )

                # ---- n-layout copy: T_n[p=(g,j), k, d] via 16 SBUF->SBUF DMAs
                tn = big.tile([128, KNBR, D], f32, tag="tn")
                for g in range(GRP):
                    nc.sync.dma_start(
                        out=tn[g * PTS_G:(g + 1) * PTS_G, :, :],
                        in_=t2[:, g, :],
                    )

                # ---- queries: Qw = src_tile @ Wqk  (Wqk includes 1/sqrt(d))
                s_sb = small.tile([128, D], f32, tag="s")
                nc.sync.dma_start(out=s_sb, in_=src_h[t * 128:(t + 1) * 128, :])
                st_ps = ps.tile([128, D], f32, tag="pss")
                nc.tensor.transpose(st_ps, s_sb, iden_sb)
                st_sb = small.tile([128, D], f32, tag="st")
                nc.scalar.copy(st_sb, st_ps)
                qw_ps = ps.tile([128, D], f32, tag="pss")
                nc.tensor.matmul(qw_ps, lhsT=st_sb, rhs=wqk_sb, start=True, stop=True)
                qw_sb = small.tile([128, D], f32, tag="qw")
                nc.scalar.copy(qw_sb, qw_ps)

                # ---- scores: prod = T_n * Qw (bcast over k), reduce over d
                import concourse.bass as bass
                qw_b = bass.AP(
                    tensor=qw_sb.tensor,
                    offset=qw_sb.offset,
                    ap=[qw_sb.ap[0], [0, KNBR], qw_sb.ap[1]],
                )
                prod = big.tile([128, KNBR, D], f32, tag="prod")
                nc.vector.tensor_mul(prod, tn, qw_b)
                scores = small.tile([128, KNBR], f32, tag="sc")
                nc.vector.reduce_sum(scores, prod, axis=mybir.AxisListType.X)

                # ---- softmax over k (scores are small; skip max-subtraction)
                e_sb = small.tile([128, KNBR], f32, tag="e")
                nc.scalar.activation(e_sb, scores, mybir.ActivationFunctionType.Exp)
                den = small.tile([128, 1], f32, tag="den")
                nc.vector.reduce_sum(den, e_sb, axis=mybir.AxisListType.X)
                rden = small.tile([128, 1], f32, tag="rden")
                nc.vector.reciprocal(rden, den)
                attn = small.tile([128, KNBR], f32, tag="attn")
                nc.vector.scalar_tensor_tensor(
                    out=attn, in0=e_sb, scalar=rden, in1=e_sb,
                    op0=mybir.AluOpType.mult, op1=mybir.AluOpType.bypass,
                )

                # ---- transpose attn -> [k, pts] then scatter to block-diag A
                at_ps = ps.tile([KNBR, 128], f32, tag="psat")
                nc.tensor.transpose(at_ps, attn, iden_sb)
                at_sb = small.tile([KNBR, 128], f32, tag="at")
                nc.scalar.copy(at_sb, at_ps)

                a_cat = small.tile([128, GRP, PTS_G], f32, tag="acat")
                nc.gpsimd.memset(a_cat, 0.0)
                at_v = at_sb.rearrange("k (g j) -> k g j", j=PTS_G)
                for j in range(PTS_G):
                    nc.sync.dma_start(
                        out=a_cat[16 * j:16 * (j + 1), :, j],
                        in_=at_v[:, :, j],
                    )

                # ---- ctx^T via 16 small matmuls: ctxT[:, g-block] = T2g.T @ A_g
                ctxt_ps = ps_ctx.tile([D, 128], f32, tag="psctx")
                for g in range(GRP):
                    nc.tensor.matmul(
                        ctxt_ps[:, g * PTS_G:(g + 1) * PTS_G],
                        lhsT=t2[:, g, :], rhs=a_cat[:, g, :],
                        start=True, stop=True,
                    )
                ctxt_sb = small.tile([D, 128], f32, tag="ctxt")
                nc.scalar.copy(ctxt_sb, ctxt_ps)

                # ---- output projection: out = ctx @ Wv.T
                out_ps = ps.tile([128, D], f32, tag="pss")
                nc.tensor.matmul(out_ps, lhsT=ctxt_sb, rhs=wvt_sb, start=True, stop=True)
                out_sb = small.tile([128, D], f32, tag="outsb")
                nc.scalar.copy(out_sb, out_ps)
                nc.sync.dma_start(out=out_h[t * 128:(t + 1) * 128, :], in_=out_sb)

    nc.compile()
    return nc


def kernel(src, tgt, Wq, Wk, Wv):
    from concourse.bass_utils import run_bass_kernel_spmd

    src = np.ascontiguousarray(src, dtype=np.float32)
    tgt = np.ascontiguousarray(tgt, dtype=np.float32)

    scale = 1.0 / math.sqrt(D)
    wqk = (Wq.astype(np.float64).T @ Wk.astype(np.float64) * scale).astype(np.float32)
    wvt = np.ascontiguousarray(Wv.astype(np.float32).T)
    iden = np.eye(D, dtype=np.float32)

    src_f = src.reshape(PTS_TOTAL, D)
    tgt_f = tgt.reshape(PTS_TOTAL * KNBR, D)

    if "nc" not in _cached:
        _cached["nc"] = _build_program()
    nc = _cached["nc"]

    in_maps = []
    for c in range(NCORES):
        p0, p1 = c * PTS_CORE, (c + 1) * PTS_CORE
        in_maps.append({
            "src_sh": np.ascontiguousarray(src_f[p0:p1]),
            "tgt_sh": np.ascontiguousarray(tgt_f[p0 * KNBR:p1 * KNBR]),
            "wqk": wqk,
            "wvt": wvt,
            "iden": iden,
        })

    _cached["in_maps"] = in_maps
    res = run_bass_kernel_spmd(nc, in_maps, core_ids=list(range(NCORES)))
    out = np.concatenate([r["out_sh"] for r in res.results], axis=0)
    return out.reshape(B, N, D).astype(np.float32)


def __getattr__(name):
    if name == "_last_in_maps":
        return _cached.get("in_maps")
    raise AttributeError(name)
